# revision 21
# baseline (speedup 1.0000x reference)
"""Trainium2 Bass kernel for a 3-layer dense transformer (BigramModel).

Contract: kernel(**inputs) takes the FULL unsharded numpy inputs (as produced
by setup_inputs) and returns the full [B, T, V] float32 logits. Internally the
batch dim B=128 is sharded 16-per-core across 8 NeuronCores (pure data
parallelism, weights replicated), one Bass/Tile NEFF run via
run_bass_kernel_spmd.

Layout strategy on device (per core, 16 seqs x 256 tok = 4096 tokens):
  - residual h: token-major fp32 SBUF tiles [128, 384] x 32 (persistent)
  - LayerNorm: DVE bn_stats/bn_aggr; rstd = exp(-0.5*ln(var+eps)); gamma/beta
    fold into adjacent weights. A single explicit ACT-table load
    (natural_log_exp_and_others) serves ln/exp/relu/identity with zero
    mid-kernel ACT_TABLE_LOADs.
  - ALL t<->e layout flips run on the XBAR DMA-transpose engine, batched one
    [128, 4x384] -> [128, 4, 3, 128] op per LN block / attention block, all
    issued from the Sync engine (the xbar is a serial resource: concurrent
    transposes from two queues corrupt - verified on HW). PE transposes and
    their DVE evacuation copies are gone entirely.
  - matmuls in bf16 (fp32 PSUM accumulation).
  - attention: scores for all 6 heads of one (seq, key-chunk) go to a single
    3-bank PSUM tile [128, 6, 256] via 6 matmuls (head pairs run concurrently
    on the PE via base-partition col/row groups), evacuated by ONE wide exp;
    probs are masked multiplicatively after exp. o accumulates per-query-tile
    in a [128, H, 65] PSUM tile with ones-augmented V supplying the softmax
    denominators; evacuation fuses the reciprocal multiply.
  - proj and the MLP second linear produce TOKEN-major outputs directly
    (activation^T chunks as the stationary operand) so the residual add
    consumes PSUM straight.
  - all-zero biases (this model instance) are detected on the host and their
    adds/b-matmuls are skipped entirely.
"""

import numpy as np
import ml_dtypes

BF16 = ml_dtypes.bfloat16

P = 128
T = 256
E = 384
V = 65
H = 6
HS = 64
FF = 1536
L = 3
NCORES = 8
BPC = 16              # sequences per core
TOK = BPC * T         # 4096 tokens per core
NT = TOK // P         # 32 token tiles
NB = TOK // 512       # 8 blocks of 512 tokens (2 seqs)
ECH = E // P          # 3
FCH = FF // P         # 12

_NC_CACHE = {}


def _build_nc(flags):
    """Build + compile the Bass program. flags = (bqk_nz, bv_nz, bp_nz, b1_nz, b2_nz)."""
    import concourse.bacc as bacc
    import concourse.mybir as mybir
    import concourse.tile as tile

    dt = mybir.dt
    f32 = dt.float32
    bf = dt.bfloat16
    Alu = mybir.AluOpType
    Act = mybir.ActivationFunctionType

    import os
    DBG_NO_DMAT = bool(os.environ.get("KBG_NO_DMAT"))
    DBG_NO_WEXP = bool(os.environ.get("KBG_NO_WEXP"))
    DBG_NO_TBL = bool(os.environ.get("KBG_NO_TBL"))
    DBG_BASE_SC = bool(os.environ.get("KBG_BASE_SC"))

    nc = bacc.Bacc("TRN2", target_bir_lowering=False, debug=False, num_devices=1)

    # ---- DRAM tensors (shapes match SBUF layouts; host pre-arranges) ----
    D = {}
    D["oh"] = nc.dram_tensor("oh", [V, TOK], bf, kind="ExternalInput")
    D["te"] = nc.dram_tensor("te", [V, E], bf, kind="ExternalInput")
    D["pos"] = nc.dram_tensor("pos", [P, 2, E], f32, kind="ExternalInput")
    D["mask"] = nc.dram_tensor("mask", [P, 2 * P], bf, kind="ExternalInput")
    for l in range(L):
        for w in ("wq", "wk", "wv", "wproj"):
            D[f"{w}{l}"] = nc.dram_tensor(f"{w}{l}", [P, ECH, E], bf, kind="ExternalInput")
        D[f"bq{l}"] = nc.dram_tensor(f"bq{l}", [P, ECH], f32, kind="ExternalInput")
        D[f"bk{l}"] = nc.dram_tensor(f"bk{l}", [P, ECH], f32, kind="ExternalInput")
        D[f"w1{l}"] = nc.dram_tensor(f"w1{l}", [P, ECH, FF], bf, kind="ExternalInput")
        D[f"b1c{l}"] = nc.dram_tensor(f"b1c{l}", [P, FCH], f32, kind="ExternalInput")
        D[f"w2{l}"] = nc.dram_tensor(f"w2{l}", [P, FCH, E], bf, kind="ExternalInput")
        D[f"bvrow{l}"] = nc.dram_tensor(f"bvrow{l}", [1, E], bf, kind="ExternalInput")
        D[f"bprow{l}"] = nc.dram_tensor(f"bprow{l}", [1, E], bf, kind="ExternalInput")
        D[f"b2row{l}"] = nc.dram_tensor(f"b2row{l}", [1, E], bf, kind="ExternalInput")
    D["wout"] = nc.dram_tensor("wout", [P, ECH, V], bf, kind="ExternalInput")
    D["boutc"] = nc.dram_tensor("boutc", [V, 1], f32, kind="ExternalInput")
    D["logT"] = nc.dram_tensor("logT", [V, TOK], f32, kind="ExternalOutput")

    bqk_nz, bv_nz, bp_nz, b1_nz, b2_nz = flags

    with tile.TileContext(nc) as tc:
        import contextlib

        with contextlib.ExitStack() as ctx:
            # one table load serving ln/exp/relu/identity for the whole kernel
            if not DBG_NO_TBL:
                nc.scalar.add_instruction(
                    mybir.InstLoadActFuncSet(
                        name="I-acttbl", ins=[], outs=[], act_func_set_id=6
                    )
                )

            const = ctx.enter_context(tc.tile_pool(name="const", bufs=1))
            wpool = ctx.enter_context(tc.tile_pool(name="wpool", bufs=2))
            act = ctx.enter_context(tc.tile_pool(name="act", bufs=4))
            act2 = ctx.enter_context(tc.tile_pool(name="act2", bufs=2))
            act1 = ctx.enter_context(tc.tile_pool(name="act1", bufs=1))
            ps_mm = ctx.enter_context(tc.tile_pool(name="ps_mm", bufs=3 if DBG_NO_DMAT else 4, space="PSUM"))
            ps_sc = None if DBG_BASE_SC else ctx.enter_context(tc.tile_pool(name="ps_sc", bufs=1, space="PSUM"))
            ps_o = ctx.enter_context(tc.tile_pool(name="ps_o", bufs=1, space="PSUM"))

            def load_const(name, shape, dtp):
                t = const.tile(shape, dtp, tag=name)
                nc.sync.dma_start(out=t[:], in_=D[name].ap())
                return t

            # pad the K=65 embedding contraction to K=128 (sub-128 partition
            # matmuls are flaky on HW); pad rows are zeroed so they add 0.
            oh_sb = const.tile([P, TOK], bf, tag="oh")
            nc.vector.memset(oh_sb[:], 0.0)
            nc.sync.dma_start(out=oh_sb[0:V, :], in_=D["oh"].ap())
            te_sb = const.tile([P, E], bf, tag="te")
            nc.vector.memset(te_sb[:], 0.0)
            nc.sync.dma_start(out=te_sb[0:V, :], in_=D["te"].ap())
            pos_sb = load_const("pos", [P, 2, E], f32)
            mask_sb = load_const("mask", [P, 2 * P], bf)
            boutc_sb = load_const("boutc", [V, 1], f32)
            ones_sb = const.tile([1, P], bf, tag="ones")
            nc.vector.memset(ones_sb[:], 1.0)
            eps_sb = const.tile([P, 1], f32, tag="eps")
            nc.vector.memset(eps_sb[:], 1e-5)

            def warm():
                """tiny matmul to keep the PE HAM clock-gate at 8/8 across
                attention stall windows; writes a scratch corner of an mm
                ring slot, never read."""
                wt = ps_mm.tile([P, 512], f32, tag="mm", name="warm")
                nc.tensor.matmul(
                    wt[0:8, 0:8], mask_sb[:, 0:8], mask_sb[:, 0:8],
                    start=True, stop=True,
                )
            if DBG_NO_DMAT:
                from concourse.masks import make_identity
                ident_sb = const.tile([P, P], bf, tag="ident")
                make_identity(nc, ident_sb[:])

            def blk_transpose(dst4, src):
                """dst4 [P,4,ECH,P] = per-tile transpose of src [P,4,E]."""
                if not DBG_NO_DMAT:
                    nc.sync.dma_start_transpose(
                        dst4[:], src[:].rearrange("p j e -> p (j e)"))
                    return
                for j in range(4):
                    for c in range(ECH):
                        tp = ps_mm.tile([P, 512], f32, tag="tp", name="tp", bufs=1)
                        tpb = tp[:, 0:64].bitcast(bf)
                        nc.tensor.transpose(
                            tpb, src[:, j, c * P:(c + 1) * P], ident_sb[:])
                        nc.vector.tensor_copy(out=dst4[:, j, c, :], in_=tpb)

            def blk_transpose2(dst4, src2, s):
                """dst4[:, 2s:2s+2] = transpose of src2 [P, 2, E] (one seq)."""
                if not DBG_NO_DMAT:
                    nc.sync.dma_start_transpose(
                        dst4[:, 2 * s:2 * s + 2],
                        src2[:].rearrange("p j e -> p (j e)"))
                    return
                for j in range(2):
                    for c in range(ECH):
                        tp = ps_mm.tile([P, 512], f32, tag="tp", name="tp", bufs=1)
                        tpb = tp[:, 0:64].bitcast(bf)
                        nc.tensor.transpose(
                            tpb, src2[:, j, c * P:(c + 1) * P], ident_sb[:])
                        nc.vector.tensor_copy(out=dst4[:, 2 * s + j, c, :], in_=tpb)

            # two persistent Vt tiles (alternating per block); the ones column
            # (col 64) is set once and never rewritten - per-block evacs only
            # touch cols 0:64.
            vt_tiles = []
            for i in range(2):
                vt = const.tile([P, 4, H, 65], bf, tag=f"Vt{i}", name=f"Vt{i}")
                nc.gpsimd.memset(vt[:, :, :, 64:65], 1.0)
                vt_tiles.append(vt)

            # persistent residual tiles
            h = []
            for i in range(NT):
                h.append(const.tile([P, E], f32, tag=f"h{i}", name=f"h{i}"))

            # ---- embedding: h = onehot.T @ tok_emb + pos ----
            def embed_emit(lo, hi):
                for i in range(lo, hi):
                    ps = ps_mm.tile([P, 512], f32, tag="mm", name="emm")
                    nc.tensor.matmul(
                        ps[:, 0:E], oh_sb[:, i * P:(i + 1) * P], te_sb[:],
                        start=True, stop=True,
                    )
                    nc.vector.tensor_add(
                        out=h[i][:], in0=ps[:, 0:E], in1=pos_sb[:, i % 2, :])

            def ln_block(i0):
                """LN of h[i0..i0+3] -> xnT [P, 4, ECH, 128] bf16 via one
                XBAR DMA transpose of the whole [P, 4, E] block."""
                xn = act2.tile([P, 4, E], bf, tag="xn")
                mv4 = act.tile([P, 4, 2], f32, tag="mv")
                rstd4 = act.tile([P, 4], f32, tag="rstd")
                for j in range(4):
                    st6 = act.tile([P, 6], f32, tag="bnst")
                    nc.vector.bn_stats(out=st6[:], in_=h[i0 + j][:])
                    nc.vector.bn_aggr(out=mv4[:, j, :], in_=st6[:])
                # rstd = exp(-0.5 * ln(var + eps))
                nc.scalar.activation(
                    out=rstd4[:], in_=mv4[:, :, 1], func=Act.Ln, bias=eps_sb[:],
                )
                nc.scalar.activation(
                    out=rstd4[:], in_=rstd4[:], func=Act.Exp, scale=-0.5,
                )
                # nmr = -mean*rstd; xn = Identity(h*rstd + nmr) on ACT
                nmr = act.tile([P, 4], f32, tag="nmr", name="nmr")
                nc.vector.scalar_tensor_tensor(
                    out=nmr[:], in0=mv4[:, :, 0], scalar=-1.0, in1=rstd4[:],
                    op0=Alu.mult, op1=Alu.mult,
                )
                for j in range(4):
                    nc.scalar.activation(
                        out=xn[:, j, :], in_=h[i0 + j][:], func=Act.Identity,
                        bias=nmr[:, j:j + 1], scale=rstd4[:, j:j + 1],
                    )
                xnT = act.tile([P, 4, ECH, P], bf, tag="xnT")
                blk_transpose(xnT, xn)
                return xnT

            def linear_fmaj(xnT, w_sb, bias_sb, fch, tag, relu=False,
                            act_evac=False):
                """feature-major out [P, fch, 512] bf16 = (W^T xn^T);
                bias per-partition (or None). relu/act_evac route evac to
                ScalarE."""
                o = (act1 if fch == FCH else act2).tile([P, fch, 512], bf, tag=tag, name=tag)
                for f in range(fch):
                    ps = ps_mm.tile([P, 512], f32, tag="mm")
                    for c in range(ECH):
                        nc.tensor.matmul(
                            ps[:], w_sb[:, c, f * P:(f + 1) * P], xnT[:, :, c, :],
                            start=(c == 0), stop=(c == ECH - 1),
                        )
                    if relu:
                        if bias_sb is not None:
                            nc.scalar.activation(
                                out=o[:, f, :], in_=ps[:], func=Act.Relu,
                                bias=bias_sb[:, f:f + 1], scale=1.0,
                            )
                        else:
                            nc.scalar.activation(
                                out=o[:, f, :], in_=ps[:], func=Act.Relu, scale=1.0,
                            )
                    elif act_evac:
                        if bias_sb is not None:
                            nc.scalar.activation(
                                out=o[:, f, :], in_=ps[:], func=Act.Identity,
                                bias=bias_sb[:, f:f + 1], scale=1.0,
                            )
                        else:
                            nc.scalar.activation(
                                out=o[:, f, :], in_=ps[:], func=Act.Identity, scale=1.0,
                            )
                    else:
                        if bias_sb is not None:
                            nc.vector.tensor_scalar_add(
                                out=o[:, f, :], in0=ps[:], scalar1=bias_sb[:, f:f + 1],
                            )
                        else:
                            nc.vector.tensor_copy(out=o[:, f, :], in_=ps[:])
                return o

            def linear_tok_resid(xT_slices, w_sb, nch, brow, i0, nj=4):
                """h[i0+j] += x @ W (+ b): token-major PSUM output via xT
                chunks as the stationary operand; residual add reads PSUM.
                xT_slices(j, c) -> stationary [P, P] AP."""
                for j in range(nj):
                    ps = ps_mm.tile([P, 512], f32, tag="mm", name="tokmm")
                    for c in range(nch):
                        nc.tensor.matmul(
                            ps[:, 0:E], xT_slices(j, c), w_sb[:, c, :],
                            start=(c == 0),
                            stop=(c == nch - 1 and brow is None),
                        )
                    if brow is not None:
                        nc.tensor.matmul(
                            ps[:, 0:E], ones_sb[:], brow[:], start=False, stop=True,
                        )
                    nc.vector.tensor_add(
                        out=h[i0 + j][:], in0=h[i0 + j][:], in1=ps[:, 0:E])

            def load_w(name, shape, dtp):
                t = wpool.tile(shape, dtp, tag=name[:-1])  # tag without layer idx
                nc.sync.dma_start(out=t[:], in_=D[name].ap())
                return t

            # ---- transformer layers (software-pipelined emission) ----
            W = {}

            def load_layer(l):
                W[l] = dict(
                    wq=load_w(f"wq{l}", [P, ECH, E], bf),
                    wk=load_w(f"wk{l}", [P, ECH, E], bf),
                    wv=load_w(f"wv{l}", [P, ECH, E], bf),
                    wproj=load_w(f"wproj{l}", [P, ECH, E], bf),
                    bq=load_w(f"bq{l}", [P, ECH], f32) if bqk_nz[l] else None,
                    bk=load_w(f"bk{l}", [P, ECH], f32) if bqk_nz[l] else None,
                    w1=load_w(f"w1{l}", [P, ECH, FF], bf),
                    b1c=load_w(f"b1c{l}", [P, FCH], f32) if b1_nz[l] else None,
                    w2=load_w(f"w2{l}", [P, FCH, E], bf),
                    bvrow=load_w(f"bvrow{l}", [1, E], bf) if bv_nz[l] else None,
                    bprow=load_w(f"bprow{l}", [1, E], bf) if bp_nz[l] else None,
                    b2row=load_w(f"b2row{l}", [1, E], bf) if b2_nz[l] else None,
                )

            def qkv_emit(l, b, xnT):
                Wl = W[l]
                wq, wk, wv = Wl["wq"], Wl["wk"], Wl["wv"]
                bq, bk, bvrow = Wl["bq"], Wl["bk"], Wl["bvrow"]
                QT = linear_fmaj(xnT, wq, bq, ECH, "QT", act_evac=True)
                KT = linear_fmaj(xnT, wk, bk, ECH, "KT", act_evac=True)
                # V token-major, ones-augmented: [P, 4, H, 65] (col 64 preset)
                Vt = vt_tiles[b % 2]
                for j in range(4):
                    ps = ps_mm.tile([P, 512], f32, tag="mm")
                    for c in range(ECH):
                        nc.tensor.matmul(
                            ps[:, 0:E], xnT[:, j, c, :], wv[:, c, :],
                            start=(c == 0),
                            stop=(c == ECH - 1 and bvrow is None),
                        )
                    if bvrow is not None:
                        nc.tensor.matmul(
                            ps[:, 0:E], ones_sb[:], bvrow[:], start=False, stop=True,
                        )
                    nc.vector.tensor_copy(
                        out=Vt[:, j, :, 0:64],
                        in_=ps[:, 0:E].rearrange("p (h d) -> p h d", h=H),
                    )
                return QT, KT, Vt

            def attn_emit(l, b, qkv, ln_next=None):
                Wl = W[l]
                wproj, bprow = Wl["wproj"], Wl["bprow"]
                i0 = 4 * b
                QT, KT, Vt = qkv

                oT = act2.tile([P, 4, ECH, P], bf, tag="oT")
                for s in range(2):      # the 2 sequences in this block
                    onorm = act2.tile([P, 2, E], bf, tag="onorm", bufs=4,
                                      name="onorm")
                    tb = s * 256        # col offset within the 512 block
                    probs = act2.tile([P, 2, H, 256], bf, tag="probs")
                    for st in range(2):  # s_tile (128 keys each)
                        tlo = 128 if st == 1 else 0
                        warm()
                        if DBG_BASE_SC:
                            for hh in range(H):
                                c, off = divmod(hh * HS, P)
                                scb = ps_mm.tile([P, 512], f32, tag="mm", name="scb")
                                nc.tensor.matmul(
                                    scb[:, 0:256 - tlo],
                                    KT[off:off + HS, c, tb + st * P: tb + (st + 1) * P],
                                    QT[off:off + HS, c, tb + tlo: tb + 256],
                                    start=True, stop=True,
                                )
                                nc.scalar.activation(
                                    out=probs[:, st, hh, tlo:256],
                                    in_=scb[:, 0:256 - tlo],
                                    func=Act.Exp, scale=float(HS) ** -0.5,
                                )
                        else:
                            # scores land in cols 0:256-tlo of slot
                            # 2*(hh%3)+hh//3: concurrently-running row-group
                            # pairs (heads 2k/2k+1 at base partitions 0/64)
                            # must write DIFFERENT psum banks, and matmul
                            # psum writes must stay 1KB-aligned (both
                            # verified on HW - violating either faults).
                            sc = ps_sc.tile([P, H, 256], f32, tag="sc", name="sc")
                            for hh in range(H):
                                c, off = divmod(hh * HS, P)
                                slot = 2 * (hh % 3) + hh // 3
                                nc.tensor.matmul(
                                    sc[:, slot, 0:256 - tlo],
                                    KT[off:off + HS, c, tb + st * P: tb + (st + 1) * P],
                                    QT[off:off + HS, c, tb + tlo: tb + 256],
                                    start=True, stop=True,
                                )
                            # one wide exp for all 6 heads of this
                            # key-chunk; probs stays in SLOT order (the mask
                            # is head-agnostic; o-matmuls index by slot)
                            if DBG_NO_WEXP or st == 1:
                                # st=1 reads are strided across psum banks -
                                # ACT faults on that (HW); per-head reads
                                # stay within a bank.
                                for sl in range(H):
                                    nc.scalar.activation(
                                        out=probs[:, st, sl, tlo:256],
                                        in_=sc[:, sl, 0:256 - tlo],
                                        func=Act.Exp, scale=float(HS) ** -0.5,
                                    )
                            else:
                                nc.scalar.activation(
                                    out=probs[:, st, :, tlo:256],
                                    in_=sc[:, :, 0:256 - tlo],
                                    func=Act.Exp, scale=float(HS) ** -0.5,
                                )
                        if st == 0:
                            nc.vector.tensor_tensor(
                                out=probs[:, 0], in0=probs[:, 0],
                                in1=mask_sb[:, None, :].to_broadcast((P, H, 256)),
                                op=Alu.mult,
                            )
                        else:
                            nc.vector.tensor_tensor(
                                out=probs[:, 1, :, P:256],
                                in0=probs[:, 1, :, P:256],
                                in1=mask_sb[:, None, 0:P].to_broadcast((P, H, P)),
                                op=Alu.mult,
                            )
                    # o-matmuls: all heads into one [P, H, 65] PSUM tile;
                    # the two key chunks accumulate in PSUM; evac fuses the
                    # softmax normalization via one recip + one broadcast mult.
                    for tt in range(2):  # query tiles of this seq
                        warm()
                        osum = ps_o.tile([P, H, 65], f32, tag="osum", name="osum")
                        for hh in range(H):
                            psl = 2 * (hh % 3) + hh // 3 if not DBG_BASE_SC else hh
                            nc.tensor.matmul(
                                osum[:, hh, :],
                                probs[:, 0, psl, tt * P:(tt + 1) * P],
                                Vt[:, 2 * s, hh, :],
                                start=True, stop=(tt == 0),
                            )
                            if tt == 1:
                                nc.tensor.matmul(
                                    osum[:, hh, :],
                                    probs[:, 1, psl, P:2 * P],
                                    Vt[:, 2 * s + 1, hh, :],
                                    start=False, stop=True,
                                )
                        rec = act.tile([P, H], f32, tag="rec", name="rec")
                        nc.vector.reciprocal(out=rec[:], in_=osum[:, :, 64])
                        nc.vector.tensor_tensor(
                            out=onorm[:, tt].rearrange("p (h d) -> p h d", h=H),
                            in0=osum[:, :, 0:64],
                            in1=rec[:, :, None].to_broadcast((P, H, HS)),
                            op=Alu.mult,
                        )
                    # per-seq transpose; seq 0's proj issues while seq 1's
                    # attention runs; seq 1's proj is DEFERRED so next-stage
                    # QKV can sit before it in the PE queue and fill the
                    # oT-transpose wait.
                    blk_transpose2(oT, onorm, s)
                    if s == 0:
                        linear_tok_resid(
                            lambda j, c: oT[:, j, c, :], wproj, ECH, bprow,
                            i0, nj=2)
                        if ln_next is not None:
                            # next stage's LN emits mid-attention: its
                            # DVE/ACT chain overlaps seq 1, and its xnT
                            # transpose queues on Sync BEFORE seq 1's oT.
                            attn_emit.xnT_next = ln_next()
                warm()
                def proj_s1():
                    linear_tok_resid(
                        lambda j, c: oT[:, 2 + j, c, :], wproj, ECH, bprow,
                        i0 + 2, nj=2)
                return proj_s1

            def mlp_emit(l, b):
                i0 = 4 * b
                xnT2 = ln_block(i0)
                aT = linear_fmaj(xnT2, W[l]["w1"], W[l]["b1c"], FCH, "aT",
                                 relu=True)
                linear_tok_resid(
                    lambda j, c: aT[:, c, j * P:(j + 1) * P], W[l]["w2"], FCH,
                    W[l]["b2row"], i0)

            wout = wpool.tile([P, ECH, V], bf, tag="wout")
            nc.sync.dma_start(out=wout[:], in_=D["wout"].ap())

            def final_emit(b, xnfT):
                ps = ps_mm.tile([P, 512], f32, tag="mm")
                for c in range(ECH):
                    nc.tensor.matmul(
                        ps[0:V, :], wout[:, c, :], xnfT[:, :, c, :],
                        start=(c == 0), stop=(c == ECH - 1),
                    )
                lt = act2.tile([V, 512], f32, tag="lt")
                nc.vector.tensor_scalar_add(out=lt[:], in0=ps[0:V, :], scalar1=boutc_sb[:])
                nc.sync.dma_start(
                    out=D["logT"].ap()[:, b * 512:(b + 1) * 512], in_=lt[:],
                )

            # stage pipeline: LN for stage i+1 is emitted during stage i,
            # and stage i's MLP trails one stage behind its attention, so
            # the LN chains + DMA transposes hide under PE-heavy stretches.
            load_layer(0)
            if L > 1:
                load_layer(1)
            stages = [(l, b) for l in range(L) for b in range(NB)]
            stages += [(L, b) for b in range(NB)]      # final LN + unembed
            # prologue: embed block 0, start its LN, then the rest
            embed_emit(0, 4)
            xnT_pre = ln_block(0)
            embed_emit(4, NT)
            # per stage, emission (= scheduler priority + Sync-queue order)
            # follows readiness: next-stage LN first (its DMA transpose must
            # not sit behind this stage's late oT transpose on the serial
            # Sync queue), then the trailing MLP as PE fill work, then the
            # current attention (oT transpose last).
            qkv_pre = qkv_emit(0, 0, xnT_pre)
            for idx, (l, b) in enumerate(stages):
                attn_emit.xnT_next = None
                ln_next = (
                    (lambda nb=stages[idx + 1][1]: ln_block(4 * nb))
                    if idx + 1 < len(stages) else None
                )
                proj_s1 = None
                if l < L:
                    proj_s1 = attn_emit(l, b, qkv_pre, ln_next)
                else:
                    final_emit(b, xnT_pre)
                if attn_emit.xnT_next is not None:
                    xnT_next = attn_emit.xnT_next
                elif ln_next is not None:
                    xnT_next = ln_next()
                else:
                    xnT_next = None
                # next stage's QKV fills this stage's oT wait, BEFORE the
                # deferred seq-1 proj in the PE queue
                if idx + 1 < len(stages) and stages[idx + 1][0] < L:
                    nl, nb2 = stages[idx + 1]
                    qkv_pre = qkv_emit(nl, nb2, xnT_next)
                if proj_s1 is not None:
                    proj_s1()
                if idx > 0 and stages[idx - 1][0] < L:
                    pl, pb = stages[idx - 1]
                    mlp_emit(pl, pb)
                    if pb == NB - 1 and pl + 2 < L:
                        load_layer(pl + 2)
                xnT_pre = xnT_next
            pl, pb = stages[-1]
            if pl < L:
                mlp_emit(pl, pb)

    nc.compile()
    return nc


def _prep_shared(inp):
    """Host-side weight prep: layout rearrangement + LN gamma/beta folding."""
    sh = {}

    def f32(x):
        return np.asarray(x, np.float32)

    sh["te"] = np.asarray(f32(inp["tok_emb"]), BF16)                      # [V,E]
    sh["pos"] = np.ascontiguousarray(
        f32(inp["pos_emb"]).reshape(2, P, E).transpose(1, 0, 2))          # [P,2,E]
    m = np.concatenate(
        [np.triu(np.ones((P, P), np.float32)), np.ones((P, P), np.float32)], axis=1)
    sh["mask"] = np.asarray(m, BF16)                                      # [P,256]

    def tile3(w, fdim):  # [E, fdim] -> [P, ECH, fdim]
        return np.ascontiguousarray(w.reshape(ECH, P, fdim).transpose(1, 0, 2))

    def col(b, nch):  # [nch*P] -> [P, nch]
        return np.ascontiguousarray(b.reshape(nch, P).T)

    bqk_nz, bv_nz, bp_nz, b1_nz, b2_nz = [], [], [], [], []
    for l in range(L):
        g1, b1_ = f32(inp["ln1_g"][l]), f32(inp["ln1_b"][l])
        g2, b2_ = f32(inp["ln2_g"][l]), f32(inp["ln2_b"][l])
        wq = f32(inp["Wq"][l]).transpose(1, 0, 2).reshape(E, E)   # head-major cols
        wk = f32(inp["Wk"][l]).transpose(1, 0, 2).reshape(E, E)
        wv = f32(inp["Wv"][l]).transpose(1, 0, 2).reshape(E, E)
        sh[f"wq{l}"] = np.asarray(tile3(g1[:, None] * wq, E), BF16)
        sh[f"wk{l}"] = np.asarray(tile3(g1[:, None] * wk, E), BF16)
        sh[f"wv{l}"] = np.asarray(tile3(g1[:, None] * wv, E), BF16)
        bq = wq.T @ b1_
        bk = wk.T @ b1_
        sh[f"bq{l}"] = col(bq, ECH)
        sh[f"bk{l}"] = col(bk, ECH)
        bqk_nz.append(bool(np.any(bq != 0) or np.any(bk != 0)))
        bv = wv.T @ b1_
        sh[f"bvrow{l}"] = np.asarray(bv[None, :], BF16)
        bv_nz.append(bool(np.any(bv != 0)))
        wp = f32(inp["Wproj"][l])
        sh[f"wproj{l}"] = np.asarray(tile3(wp, E), BF16)
        bp = f32(inp["bproj"][l])
        sh[f"bprow{l}"] = np.asarray(bp[None, :], BF16)
        bp_nz.append(bool(np.any(bp != 0)))
        w1 = f32(inp["W1"][l])
        sh[f"w1{l}"] = np.asarray(tile3(g2[:, None] * w1, FF), BF16)
        b1c = f32(inp["b1"][l]) + w1.T @ b2_
        sh[f"b1c{l}"] = col(b1c, FCH)
        b1_nz.append(bool(np.any(b1c != 0)))
        w2 = f32(inp["W2"][l])
        sh[f"w2{l}"] = np.asarray(
            w2.reshape(FCH, P, E).transpose(1, 0, 2), BF16)
        b2r = f32(inp["b2"][l])
        sh[f"b2row{l}"] = np.asarray(b2r[None, :], BF16)
        b2_nz.append(bool(np.any(b2r != 0)))

    gf, bf_ = f32(inp["lnf_g"]), f32(inp["lnf_b"])
    wo = f32(inp["Wout"])
    sh["wout"] = np.asarray(tile3(gf[:, None] * wo, V), BF16)
    sh["boutc"] = (f32(inp["bout"]) + wo.T @ bf_).reshape(V, 1)
    flags = (tuple(bqk_nz), tuple(bv_nz), tuple(bp_nz), tuple(b1_nz), tuple(b2_nz))
    return sh, flags


def _onehot(xc):
    """xc: [BPC, T] ints -> [V, TOK] bf16 one-hot (feature-major)."""
    xf = np.asarray(xc, np.int64).reshape(-1)
    oh = np.zeros((V, TOK), np.float32)
    oh[xf, np.arange(TOK)] = 1.0
    return np.asarray(oh, BF16)


def _get_nc(flags):
    import os
    if os.environ.get("KBG_ALLBIAS"):
        flags = tuple(tuple(True for _ in f) for f in flags)
    if flags not in _NC_CACHE:
        _NC_CACHE[flags] = _build_nc(flags)
    return _NC_CACHE[flags]


def make_in_maps(inputs):
    sh, flags = _prep_shared(inputs)
    x = np.asarray(inputs["x"])
    in_maps = []
    for c in range(NCORES):
        m = dict(sh)
        m["oh"] = _onehot(x[c * BPC:(c + 1) * BPC])
        in_maps.append(m)
    return in_maps, flags


def kernel(**inputs):
    import os
    from concourse.bass_utils import run_bass_kernel_spmd

    in_maps, flags = make_in_maps(inputs)
    nc = _get_nc(flags)
    kw = {}
    if os.environ.get("BASS_TRACE"):
        d = os.environ.get("BASS_TRACE_DIR", "/tmp/bass_trace")
        os.makedirs(d, exist_ok=True)
        kw["tmpdir"] = d
    res = run_bass_kernel_spmd(nc, in_maps, list(range(NCORES)), **kw)
    kernel._last = res
    outs = []
    for c in range(NCORES):
        lt = np.asarray(res.results[c]["logT"], np.float32)   # [V, TOK]
        outs.append(np.ascontiguousarray(lt.T).reshape(BPC, T, V))
    return np.concatenate(outs, axis=0)


kernel._last = None


# revision 23
# speedup vs baseline: 1.0679x; 1.0679x over previous
"""Trainium2 Bass kernel for a 3-layer dense transformer (BigramModel).

Contract: kernel(**inputs) takes the FULL unsharded numpy inputs (as produced
by setup_inputs) and returns the full [B, T, V] float32 logits. Internally the
batch dim B=128 is sharded 16-per-core across 8 NeuronCores (pure data
parallelism, weights replicated), one Bass/Tile NEFF run via
run_bass_kernel_spmd.

Layout strategy on device (per core, 16 seqs x 256 tok = 4096 tokens):
  - residual h: token-major fp32 SBUF tiles [128, 384] x 32 (persistent)
  - LayerNorm: DVE bn_stats/bn_aggr; rstd = exp(-0.5*ln(var+eps)); gamma/beta
    fold into adjacent weights. A single explicit ACT-table load
    (natural_log_exp_and_others) serves ln/exp/relu/identity with zero
    mid-kernel ACT_TABLE_LOADs.
  - ALL t<->e layout flips run on the XBAR DMA-transpose engine, batched one
    [128, 4x384] -> [128, 4, 3, 128] op per LN block / attention block, all
    issued from the Sync engine (the xbar is a serial resource: concurrent
    transposes from two queues corrupt - verified on HW). PE transposes and
    their DVE evacuation copies are gone entirely.
  - matmuls in bf16 (fp32 PSUM accumulation).
  - attention: scores for all 6 heads of one (seq, key-chunk) go to a single
    3-bank PSUM tile [128, 6, 256] via 6 matmuls (head pairs run concurrently
    on the PE via base-partition col/row groups), evacuated by ONE wide exp;
    probs are masked multiplicatively after exp. o accumulates per-query-tile
    in a [128, H, 65] PSUM tile with ones-augmented V supplying the softmax
    denominators; evacuation fuses the reciprocal multiply.
  - proj and the MLP second linear produce TOKEN-major outputs directly
    (activation^T chunks as the stationary operand) so the residual add
    consumes PSUM straight.
  - all-zero biases (this model instance) are detected on the host and their
    adds/b-matmuls are skipped entirely.
"""

import numpy as np
import ml_dtypes

BF16 = ml_dtypes.bfloat16

P = 128
T = 256
E = 384
V = 65
H = 6
HS = 64
FF = 1536
L = 3
NCORES = 8
BPC = 16              # sequences per core
TOK = BPC * T         # 4096 tokens per core
NT = TOK // P         # 32 token tiles
NB = TOK // 512       # 8 blocks of 512 tokens (2 seqs)
ECH = E // P          # 3
FCH = FF // P         # 12

_NC_CACHE = {}


def _build_nc(flags):
    """Build + compile the Bass program. flags = (bqk_nz, bv_nz, bp_nz, b1_nz, b2_nz)."""
    import concourse.bacc as bacc
    import concourse.mybir as mybir
    import concourse.tile as tile

    dt = mybir.dt
    f32 = dt.float32
    bf = dt.bfloat16
    Alu = mybir.AluOpType
    Act = mybir.ActivationFunctionType

    import os
    DBG_NO_DMAT = bool(os.environ.get("KBG_NO_DMAT"))
    DBG_NO_WEXP = bool(os.environ.get("KBG_NO_WEXP"))
    DBG_NO_TBL = bool(os.environ.get("KBG_NO_TBL"))
    DBG_BASE_SC = bool(os.environ.get("KBG_BASE_SC"))

    nc = bacc.Bacc("TRN2", target_bir_lowering=False, debug=False, num_devices=1)

    # ---- DRAM tensors (shapes match SBUF layouts; host pre-arranges) ----
    D = {}
    D["oh"] = nc.dram_tensor("oh", [V, TOK], bf, kind="ExternalInput")
    D["te"] = nc.dram_tensor("te", [V, E], bf, kind="ExternalInput")
    D["pos"] = nc.dram_tensor("pos", [P, 2, E], f32, kind="ExternalInput")
    D["mask"] = nc.dram_tensor("mask", [P, 2 * P], bf, kind="ExternalInput")
    for l in range(L):
        for w in ("wq", "wk", "wv", "wproj"):
            D[f"{w}{l}"] = nc.dram_tensor(f"{w}{l}", [P, ECH, E], bf, kind="ExternalInput")
        D[f"bq{l}"] = nc.dram_tensor(f"bq{l}", [P, ECH], f32, kind="ExternalInput")
        D[f"bk{l}"] = nc.dram_tensor(f"bk{l}", [P, ECH], f32, kind="ExternalInput")
        D[f"w1{l}"] = nc.dram_tensor(f"w1{l}", [P, ECH, FF], bf, kind="ExternalInput")
        D[f"b1c{l}"] = nc.dram_tensor(f"b1c{l}", [P, FCH], f32, kind="ExternalInput")
        D[f"w2{l}"] = nc.dram_tensor(f"w2{l}", [P, FCH, E], bf, kind="ExternalInput")
        D[f"bvrow{l}"] = nc.dram_tensor(f"bvrow{l}", [1, E], bf, kind="ExternalInput")
        D[f"bprow{l}"] = nc.dram_tensor(f"bprow{l}", [1, E], bf, kind="ExternalInput")
        D[f"b2row{l}"] = nc.dram_tensor(f"b2row{l}", [1, E], bf, kind="ExternalInput")
    D["wout"] = nc.dram_tensor("wout", [P, ECH, V], bf, kind="ExternalInput")
    D["boutc"] = nc.dram_tensor("boutc", [V, 1], f32, kind="ExternalInput")
    D["logT"] = nc.dram_tensor("logT", [V, TOK], f32, kind="ExternalOutput")

    bqk_nz, bv_nz, bp_nz, b1_nz, b2_nz = flags

    with tile.TileContext(nc) as tc:
        import contextlib

        with contextlib.ExitStack() as ctx:
            # one table load serving ln/exp/relu/identity for the whole kernel
            if not DBG_NO_TBL:
                nc.scalar.add_instruction(
                    mybir.InstLoadActFuncSet(
                        name="I-acttbl", ins=[], outs=[], act_func_set_id=6
                    )
                )

            const = ctx.enter_context(tc.tile_pool(name="const", bufs=1))
            wpool = ctx.enter_context(tc.tile_pool(name="wpool", bufs=2))
            act = ctx.enter_context(tc.tile_pool(name="act", bufs=4))
            act2 = ctx.enter_context(tc.tile_pool(name="act2", bufs=2))
            act1 = ctx.enter_context(tc.tile_pool(name="act1", bufs=1))
            ps_mm = ctx.enter_context(tc.tile_pool(name="ps_mm", bufs=3 if DBG_NO_DMAT else 4, space="PSUM"))
            ps_sc = None if DBG_BASE_SC else ctx.enter_context(tc.tile_pool(name="ps_sc", bufs=1, space="PSUM"))
            ps_o = ctx.enter_context(tc.tile_pool(name="ps_o", bufs=1, space="PSUM"))

            def load_const(name, shape, dtp):
                t = const.tile(shape, dtp, tag=name)
                nc.sync.dma_start(out=t[:], in_=D[name].ap())
                return t

            # pad the K=65 embedding contraction to K=128 (sub-128 partition
            # matmuls are flaky on HW); pad rows are zeroed so they add 0.
            oh_sb = const.tile([P, TOK], bf, tag="oh")
            nc.vector.memset(oh_sb[:], 0.0)
            nc.sync.dma_start(out=oh_sb[0:V, :], in_=D["oh"].ap())
            te_sb = const.tile([P, E], bf, tag="te")
            nc.vector.memset(te_sb[:], 0.0)
            nc.sync.dma_start(out=te_sb[0:V, :], in_=D["te"].ap())
            pos_sb = load_const("pos", [P, 2, E], f32)
            mask_sb = load_const("mask", [P, 2 * P], bf)
            boutc_sb = load_const("boutc", [V, 1], f32)
            ones_sb = const.tile([1, P], bf, tag="ones")
            nc.vector.memset(ones_sb[:], 1.0)
            eps_sb = const.tile([P, 1], f32, tag="eps")
            nc.vector.memset(eps_sb[:], 1e-5)

            def warm():
                """tiny matmul to keep the PE HAM clock-gate at 8/8 across
                attention stall windows; writes a scratch corner of an mm
                ring slot, never read."""
                wt = ps_mm.tile([P, 512], f32, tag="mm", name="warm")
                nc.tensor.matmul(
                    wt[0:8, 0:8], mask_sb[:, 0:8], mask_sb[:, 0:8],
                    start=True, stop=True,
                )
            if DBG_NO_DMAT:
                from concourse.masks import make_identity
                ident_sb = const.tile([P, P], bf, tag="ident")
                make_identity(nc, ident_sb[:])

            def blk_transpose(dst4, src):
                """dst4 [P,4,ECH,P] = per-tile transpose of src [P,4,E]."""
                if not DBG_NO_DMAT:
                    nc.sync.dma_start_transpose(
                        dst4[:], src[:].rearrange("p j e -> p (j e)"))
                    return
                for j in range(4):
                    for c in range(ECH):
                        tp = ps_mm.tile([P, 512], f32, tag="tp", name="tp", bufs=1)
                        tpb = tp[:, 0:64].bitcast(bf)
                        nc.tensor.transpose(
                            tpb, src[:, j, c * P:(c + 1) * P], ident_sb[:])
                        nc.vector.tensor_copy(out=dst4[:, j, c, :], in_=tpb)

            def blk_transpose2(dst4, src2, s):
                """dst4[:, 2s:2s+2] = transpose of src2 [P, 2, E] (one seq)."""
                if not DBG_NO_DMAT:
                    nc.sync.dma_start_transpose(
                        dst4[:, 2 * s:2 * s + 2],
                        src2[:].rearrange("p j e -> p (j e)"))
                    return
                for j in range(2):
                    for c in range(ECH):
                        tp = ps_mm.tile([P, 512], f32, tag="tp", name="tp", bufs=1)
                        tpb = tp[:, 0:64].bitcast(bf)
                        nc.tensor.transpose(
                            tpb, src2[:, j, c * P:(c + 1) * P], ident_sb[:])
                        nc.vector.tensor_copy(out=dst4[:, 2 * s + j, c, :], in_=tpb)

            # two persistent Vt tiles (alternating per block); the ones column
            # (col 64) is set once and never rewritten - per-block evacs only
            # touch cols 0:64.
            vt_tiles = []
            for i in range(2):
                vt = const.tile([P, 4, H, 65], bf, tag=f"Vt{i}", name=f"Vt{i}")
                nc.gpsimd.memset(vt[:, :, :, 64:65], 1.0)
                vt_tiles.append(vt)

            # persistent residual tiles
            h = []
            for i in range(NT):
                h.append(const.tile([P, E], f32, tag=f"h{i}", name=f"h{i}"))

            # ---- embedding: h = onehot.T @ tok_emb + pos ----
            def embed_emit(lo, hi):
                for i in range(lo, hi):
                    ps = ps_mm.tile([P, 512], f32, tag="mm", name="emm")
                    nc.tensor.matmul(
                        ps[:, 0:E], oh_sb[:, i * P:(i + 1) * P], te_sb[:],
                        start=True, stop=True,
                    )
                    nc.vector.tensor_add(
                        out=h[i][:], in0=ps[:, 0:E], in1=pos_sb[:, i % 2, :])

            def ln_block(i0):
                """LN of h[i0..i0+3] -> xnT [P, 4, ECH, 128] bf16 via one
                XBAR DMA transpose of the whole [P, 4, E] block."""
                xn = act2.tile([P, 4, E], bf, tag="xn")
                mv4 = act.tile([P, 4, 2], f32, tag="mv")
                rstd4 = act.tile([P, 4], f32, tag="rstd")
                for j in range(4):
                    st6 = act.tile([P, 6], f32, tag="bnst")
                    nc.vector.bn_stats(out=st6[:], in_=h[i0 + j][:])
                    nc.vector.bn_aggr(out=mv4[:, j, :], in_=st6[:])
                # rstd = exp(-0.5 * ln(var + eps))
                nc.scalar.activation(
                    out=rstd4[:], in_=mv4[:, :, 1], func=Act.Ln, bias=eps_sb[:],
                )
                nc.scalar.activation(
                    out=rstd4[:], in_=rstd4[:], func=Act.Exp, scale=-0.5,
                )
                for j in range(4):
                    nc.vector.tensor_scalar(
                        out=xn[:, j, :], in0=h[i0 + j][:],
                        scalar1=mv4[:, j, 0:1], scalar2=rstd4[:, j:j + 1],
                        op0=Alu.subtract, op1=Alu.mult,
                    )
                xnT = act.tile([P, 4, ECH, P], bf, tag="xnT")
                blk_transpose(xnT, xn)
                return xnT

            def linear_fmaj(xnT, w_sb, bias_sb, fch, tag, relu=False,
                            act_evac=False):
                """feature-major out [P, fch, 512] bf16 = (W^T xn^T);
                bias per-partition (or None). relu/act_evac route evac to
                ScalarE."""
                o = (act1 if fch == FCH else act2).tile([P, fch, 512], bf, tag=tag, name=tag)
                for f in range(fch):
                    ps = ps_mm.tile([P, 512], f32, tag="mm")
                    for c in range(ECH):
                        nc.tensor.matmul(
                            ps[:], w_sb[:, c, f * P:(f + 1) * P], xnT[:, :, c, :],
                            start=(c == 0), stop=(c == ECH - 1),
                        )
                    if relu:
                        if bias_sb is not None:
                            nc.scalar.activation(
                                out=o[:, f, :], in_=ps[:], func=Act.Relu,
                                bias=bias_sb[:, f:f + 1], scale=1.0,
                            )
                        else:
                            nc.scalar.activation(
                                out=o[:, f, :], in_=ps[:], func=Act.Relu, scale=1.0,
                            )
                    elif act_evac:
                        if bias_sb is not None:
                            nc.scalar.activation(
                                out=o[:, f, :], in_=ps[:], func=Act.Identity,
                                bias=bias_sb[:, f:f + 1], scale=1.0,
                            )
                        else:
                            nc.scalar.activation(
                                out=o[:, f, :], in_=ps[:], func=Act.Identity, scale=1.0,
                            )
                    else:
                        if bias_sb is not None:
                            nc.vector.tensor_scalar_add(
                                out=o[:, f, :], in0=ps[:], scalar1=bias_sb[:, f:f + 1],
                            )
                        else:
                            nc.vector.tensor_copy(out=o[:, f, :], in_=ps[:])
                return o

            def linear_tok_resid(xT_slices, w_sb, nch, brow, i0, nj=4):
                """h[i0+j] += x @ W (+ b): token-major PSUM output via xT
                chunks as the stationary operand; residual add reads PSUM.
                xT_slices(j, c) -> stationary [P, P] AP."""
                for j in range(nj):
                    ps = ps_mm.tile([P, 512], f32, tag="mm", name="tokmm")
                    for c in range(nch):
                        nc.tensor.matmul(
                            ps[:, 0:E], xT_slices(j, c), w_sb[:, c, :],
                            start=(c == 0),
                            stop=(c == nch - 1 and brow is None),
                        )
                    if brow is not None:
                        nc.tensor.matmul(
                            ps[:, 0:E], ones_sb[:], brow[:], start=False, stop=True,
                        )
                    nc.vector.tensor_add(
                        out=h[i0 + j][:], in0=h[i0 + j][:], in1=ps[:, 0:E])

            def load_w(name, shape, dtp):
                t = wpool.tile(shape, dtp, tag=name[:-1])  # tag without layer idx
                nc.sync.dma_start(out=t[:], in_=D[name].ap())
                return t

            # ---- transformer layers (software-pipelined emission) ----
            W = {}

            def load_layer(l):
                W[l] = dict(
                    wq=load_w(f"wq{l}", [P, ECH, E], bf),
                    wk=load_w(f"wk{l}", [P, ECH, E], bf),
                    wv=load_w(f"wv{l}", [P, ECH, E], bf),
                    wproj=load_w(f"wproj{l}", [P, ECH, E], bf),
                    bq=load_w(f"bq{l}", [P, ECH], f32) if bqk_nz[l] else None,
                    bk=load_w(f"bk{l}", [P, ECH], f32) if bqk_nz[l] else None,
                    w1=load_w(f"w1{l}", [P, ECH, FF], bf),
                    b1c=load_w(f"b1c{l}", [P, FCH], f32) if b1_nz[l] else None,
                    w2=load_w(f"w2{l}", [P, FCH, E], bf),
                    bvrow=load_w(f"bvrow{l}", [1, E], bf) if bv_nz[l] else None,
                    bprow=load_w(f"bprow{l}", [1, E], bf) if bp_nz[l] else None,
                    b2row=load_w(f"b2row{l}", [1, E], bf) if b2_nz[l] else None,
                )

            def attn_emit(l, b, xnT):
                Wl = W[l]
                wq, wk, wv = Wl["wq"], Wl["wk"], Wl["wv"]
                wproj, bq, bk = Wl["wproj"], Wl["bq"], Wl["bk"]
                bvrow, bprow = Wl["bvrow"], Wl["bprow"]
                i0 = 4 * b
                QT = linear_fmaj(xnT, wq, bq, ECH, "QT", act_evac=True)
                KT = linear_fmaj(xnT, wk, bk, ECH, "KT", act_evac=True)
                # V token-major, ones-augmented: [P, 4, H, 65] (col 64 preset)
                Vt = vt_tiles[b % 2]
                for j in range(4):
                    ps = ps_mm.tile([P, 512], f32, tag="mm")
                    for c in range(ECH):
                        nc.tensor.matmul(
                            ps[:, 0:E], xnT[:, j, c, :], wv[:, c, :],
                            start=(c == 0),
                            stop=(c == ECH - 1 and bvrow is None),
                        )
                    if bvrow is not None:
                        nc.tensor.matmul(
                            ps[:, 0:E], ones_sb[:], bvrow[:], start=False, stop=True,
                        )
                    nc.vector.tensor_copy(
                        out=Vt[:, j, :, 0:64],
                        in_=ps[:, 0:E].rearrange("p (h d) -> p h d", h=H),
                    )

                onorm = act2.tile([P, 4, E], bf, tag="onorm")
                for s in range(2):      # the 2 sequences in this block
                    tb = s * 256        # col offset within the 512 block
                    probs = act2.tile([P, 2, H, 256], bf, tag="probs")
                    for st in range(2):  # s_tile (128 keys each)
                        tlo = 128 if st == 1 else 0
                        if DBG_BASE_SC:
                            for hh in range(H):
                                c, off = divmod(hh * HS, P)
                                scb = ps_mm.tile([P, 512], f32, tag="mm", name="scb")
                                nc.tensor.matmul(
                                    scb[:, 0:256 - tlo],
                                    KT[off:off + HS, c, tb + st * P: tb + (st + 1) * P],
                                    QT[off:off + HS, c, tb + tlo: tb + 256],
                                    start=True, stop=True,
                                )
                                nc.scalar.activation(
                                    out=probs[:, st, hh, tlo:256],
                                    in_=scb[:, 0:256 - tlo],
                                    func=Act.Exp, scale=float(HS) ** -0.5,
                                )
                        else:
                            # scores land in cols 0:256-tlo of slot
                            # 2*(hh%3)+hh//3: concurrently-running row-group
                            # pairs (heads 2k/2k+1 at base partitions 0/64)
                            # must write DIFFERENT psum banks, and matmul
                            # psum writes must stay 1KB-aligned (both
                            # verified on HW - violating either faults).
                            sc = ps_sc.tile([P, H, 256], f32, tag="sc", name="sc")
                            for hh in range(H):
                                c, off = divmod(hh * HS, P)
                                slot = 2 * (hh % 3) + hh // 3
                                nc.tensor.matmul(
                                    sc[:, slot, 0:256 - tlo],
                                    KT[off:off + HS, c, tb + st * P: tb + (st + 1) * P],
                                    QT[off:off + HS, c, tb + tlo: tb + 256],
                                    start=True, stop=True,
                                )
                            # one wide exp for all 6 heads of this
                            # key-chunk; probs stays in SLOT order (the mask
                            # is head-agnostic; o-matmuls index by slot)
                            if DBG_NO_WEXP or st == 1:
                                # st=1 reads are strided across psum banks -
                                # ACT faults on that (HW); per-head reads
                                # stay within a bank.
                                for sl in range(H):
                                    nc.scalar.activation(
                                        out=probs[:, st, sl, tlo:256],
                                        in_=sc[:, sl, 0:256 - tlo],
                                        func=Act.Exp, scale=float(HS) ** -0.5,
                                    )
                            else:
                                nc.scalar.activation(
                                    out=probs[:, st, :, tlo:256],
                                    in_=sc[:, :, 0:256 - tlo],
                                    func=Act.Exp, scale=float(HS) ** -0.5,
                                )
                        if st == 0:
                            nc.vector.tensor_tensor(
                                out=probs[:, 0], in0=probs[:, 0],
                                in1=mask_sb[:, None, :].to_broadcast((P, H, 256)),
                                op=Alu.mult,
                            )
                        else:
                            nc.vector.tensor_tensor(
                                out=probs[:, 1, :, P:256],
                                in0=probs[:, 1, :, P:256],
                                in1=mask_sb[:, None, 0:P].to_broadcast((P, H, P)),
                                op=Alu.mult,
                            )
                    # o-matmuls: all heads into one [P, H, 65] PSUM tile;
                    # the two key chunks accumulate in PSUM; evac fuses the
                    # softmax normalization via one recip + one broadcast mult.
                    for tt in range(2):  # query tiles of this seq
                        osum = ps_o.tile([P, H, 65], f32, tag="osum", name="osum")
                        for hh in range(H):
                            psl = 2 * (hh % 3) + hh // 3 if not DBG_BASE_SC else hh
                            nc.tensor.matmul(
                                osum[:, hh, :],
                                probs[:, 0, psl, tt * P:(tt + 1) * P],
                                Vt[:, 2 * s, hh, :],
                                start=True, stop=(tt == 0),
                            )
                            if tt == 1:
                                nc.tensor.matmul(
                                    osum[:, hh, :],
                                    probs[:, 1, psl, P:2 * P],
                                    Vt[:, 2 * s + 1, hh, :],
                                    start=False, stop=True,
                                )
                        rec = act.tile([P, H], f32, tag="rec", name="rec")
                        nc.vector.reciprocal(out=rec[:], in_=osum[:, :, 64])
                        nc.vector.tensor_tensor(
                            out=onorm[:, 2 * s + tt].rearrange("p (h d) -> p h d", h=H),
                            in0=osum[:, :, 0:64],
                            in1=rec[:, :, None].to_broadcast((P, H, HS)),
                            op=Alu.mult,
                        )
                oT = act2.tile([P, 4, ECH, P], bf, tag="oT")
                blk_transpose(oT, onorm)
                linear_tok_resid(
                    lambda j, c: oT[:, j, c, :], wproj, ECH, bprow, i0)

            def mlp_emit(l, b):
                i0 = 4 * b
                xnT2 = ln_block(i0)
                aT = linear_fmaj(xnT2, W[l]["w1"], W[l]["b1c"], FCH, "aT",
                                 relu=True)
                linear_tok_resid(
                    lambda j, c: aT[:, c, j * P:(j + 1) * P], W[l]["w2"], FCH,
                    W[l]["b2row"], i0)

            wout = wpool.tile([P, ECH, V], bf, tag="wout")
            nc.sync.dma_start(out=wout[:], in_=D["wout"].ap())

            def final_emit(b, xnfT):
                ps = ps_mm.tile([P, 512], f32, tag="mm")
                for c in range(ECH):
                    nc.tensor.matmul(
                        ps[0:V, :], wout[:, c, :], xnfT[:, :, c, :],
                        start=(c == 0), stop=(c == ECH - 1),
                    )
                lt = act2.tile([V, 512], f32, tag="lt")
                nc.vector.tensor_scalar_add(out=lt[:], in0=ps[0:V, :], scalar1=boutc_sb[:])
                nc.sync.dma_start(
                    out=D["logT"].ap()[:, b * 512:(b + 1) * 512], in_=lt[:],
                )

            # stage pipeline: LN for stage i+1 is emitted during stage i,
            # and stage i's MLP trails one stage behind its attention, so
            # the LN chains + DMA transposes hide under PE-heavy stretches.
            load_layer(0)
            if L > 1:
                load_layer(1)
            stages = [(l, b) for l in range(L) for b in range(NB)]
            stages += [(L, b) for b in range(NB)]      # final LN + unembed
            # prologue: embed block 0, start its LN, then the rest
            embed_emit(0, 4)
            xnT_pre = ln_block(0)
            embed_emit(4, NT)
            # per stage, emission (= scheduler priority + Sync-queue order)
            # follows readiness: next-stage LN first (its DMA transpose must
            # not sit behind this stage's late oT transpose on the serial
            # Sync queue), then the trailing MLP as PE fill work, then the
            # current attention (oT transpose last).
            for idx, (l, b) in enumerate(stages):
                if l < L:
                    attn_emit(l, b, xnT_pre)
                else:
                    final_emit(b, xnT_pre)
                if idx > 0 and stages[idx - 1][0] < L:
                    pl, pb = stages[idx - 1]
                    mlp_emit(pl, pb)
                    if pb == NB - 1 and pl + 2 < L:
                        load_layer(pl + 2)
                if idx + 1 < len(stages):
                    xnT_pre = ln_block(4 * stages[idx + 1][1])
            pl, pb = stages[-1]
            if pl < L:
                mlp_emit(pl, pb)

    nc.compile()
    return nc


def _prep_shared(inp):
    """Host-side weight prep: layout rearrangement + LN gamma/beta folding."""
    sh = {}

    def f32(x):
        return np.asarray(x, np.float32)

    sh["te"] = np.asarray(f32(inp["tok_emb"]), BF16)                      # [V,E]
    sh["pos"] = np.ascontiguousarray(
        f32(inp["pos_emb"]).reshape(2, P, E).transpose(1, 0, 2))          # [P,2,E]
    m = np.concatenate(
        [np.triu(np.ones((P, P), np.float32)), np.ones((P, P), np.float32)], axis=1)
    sh["mask"] = np.asarray(m, BF16)                                      # [P,256]

    def tile3(w, fdim):  # [E, fdim] -> [P, ECH, fdim]
        return np.ascontiguousarray(w.reshape(ECH, P, fdim).transpose(1, 0, 2))

    def col(b, nch):  # [nch*P] -> [P, nch]
        return np.ascontiguousarray(b.reshape(nch, P).T)

    bqk_nz, bv_nz, bp_nz, b1_nz, b2_nz = [], [], [], [], []
    for l in range(L):
        g1, b1_ = f32(inp["ln1_g"][l]), f32(inp["ln1_b"][l])
        g2, b2_ = f32(inp["ln2_g"][l]), f32(inp["ln2_b"][l])
        wq = f32(inp["Wq"][l]).transpose(1, 0, 2).reshape(E, E)   # head-major cols
        wk = f32(inp["Wk"][l]).transpose(1, 0, 2).reshape(E, E)
        wv = f32(inp["Wv"][l]).transpose(1, 0, 2).reshape(E, E)
        sh[f"wq{l}"] = np.asarray(tile3(g1[:, None] * wq, E), BF16)
        sh[f"wk{l}"] = np.asarray(tile3(g1[:, None] * wk, E), BF16)
        sh[f"wv{l}"] = np.asarray(tile3(g1[:, None] * wv, E), BF16)
        bq = wq.T @ b1_
        bk = wk.T @ b1_
        sh[f"bq{l}"] = col(bq, ECH)
        sh[f"bk{l}"] = col(bk, ECH)
        bqk_nz.append(bool(np.any(bq != 0) or np.any(bk != 0)))
        bv = wv.T @ b1_
        sh[f"bvrow{l}"] = np.asarray(bv[None, :], BF16)
        bv_nz.append(bool(np.any(bv != 0)))
        wp = f32(inp["Wproj"][l])
        sh[f"wproj{l}"] = np.asarray(tile3(wp, E), BF16)
        bp = f32(inp["bproj"][l])
        sh[f"bprow{l}"] = np.asarray(bp[None, :], BF16)
        bp_nz.append(bool(np.any(bp != 0)))
        w1 = f32(inp["W1"][l])
        sh[f"w1{l}"] = np.asarray(tile3(g2[:, None] * w1, FF), BF16)
        b1c = f32(inp["b1"][l]) + w1.T @ b2_
        sh[f"b1c{l}"] = col(b1c, FCH)
        b1_nz.append(bool(np.any(b1c != 0)))
        w2 = f32(inp["W2"][l])
        sh[f"w2{l}"] = np.asarray(
            w2.reshape(FCH, P, E).transpose(1, 0, 2), BF16)
        b2r = f32(inp["b2"][l])
        sh[f"b2row{l}"] = np.asarray(b2r[None, :], BF16)
        b2_nz.append(bool(np.any(b2r != 0)))

    gf, bf_ = f32(inp["lnf_g"]), f32(inp["lnf_b"])
    wo = f32(inp["Wout"])
    sh["wout"] = np.asarray(tile3(gf[:, None] * wo, V), BF16)
    sh["boutc"] = (f32(inp["bout"]) + wo.T @ bf_).reshape(V, 1)
    flags = (tuple(bqk_nz), tuple(bv_nz), tuple(bp_nz), tuple(b1_nz), tuple(b2_nz))
    return sh, flags


def _onehot(xc):
    """xc: [BPC, T] ints -> [V, TOK] bf16 one-hot (feature-major)."""
    xf = np.asarray(xc, np.int64).reshape(-1)
    oh = np.zeros((V, TOK), np.float32)
    oh[xf, np.arange(TOK)] = 1.0
    return np.asarray(oh, BF16)


def _get_nc(flags):
    import os
    if os.environ.get("KBG_ALLBIAS"):
        flags = tuple(tuple(True for _ in f) for f in flags)
    if flags not in _NC_CACHE:
        _NC_CACHE[flags] = _build_nc(flags)
    return _NC_CACHE[flags]


def make_in_maps(inputs):
    sh, flags = _prep_shared(inputs)
    x = np.asarray(inputs["x"])
    in_maps = []
    for c in range(NCORES):
        m = dict(sh)
        m["oh"] = _onehot(x[c * BPC:(c + 1) * BPC])
        in_maps.append(m)
    return in_maps, flags


def kernel(**inputs):
    import os
    from concourse.bass_utils import run_bass_kernel_spmd

    in_maps, flags = make_in_maps(inputs)
    nc = _get_nc(flags)
    kw = {}
    if os.environ.get("BASS_TRACE"):
        d = os.environ.get("BASS_TRACE_DIR", "/tmp/bass_trace")
        os.makedirs(d, exist_ok=True)
        kw["tmpdir"] = d
    res = None
    last_err = None
    for attempt in range(3):
        try:
            res = run_bass_kernel_spmd(nc, in_maps, list(range(NCORES)), **kw)
            # force materialization so a flaky device fault surfaces here
            for c in range(NCORES):
                np.asarray(res.results[c]["logT"])
            break
        except Exception as e:          # intermittent device-side fault: retry
            last_err = e
            res = None
    if res is None:
        raise last_err
    kernel._last = res
    outs = []
    for c in range(NCORES):
        lt = np.asarray(res.results[c]["logT"], np.float32)   # [V, TOK]
        outs.append(np.ascontiguousarray(lt.T).reshape(BPC, T, V))
    return np.concatenate(outs, axis=0)


kernel._last = None


# revision 24
# speedup vs baseline: 1.0695x; 1.0015x over previous
"""Trainium2 Bass kernel for a 3-layer dense transformer (BigramModel).

Contract: kernel(**inputs) takes the FULL unsharded numpy inputs (as produced
by setup_inputs) and returns the full [B, T, V] float32 logits. Internally the
batch dim B=128 is sharded 16-per-core across 8 NeuronCores (pure data
parallelism, weights replicated), one Bass/Tile NEFF run via
run_bass_kernel_spmd.

Layout strategy on device (per core, 16 seqs x 256 tok = 4096 tokens):
  - residual h: token-major fp32 SBUF tiles [128, 384] x 32 (persistent)
  - LayerNorm: DVE bn_stats/bn_aggr; rstd = exp(-0.5*ln(var+eps)); gamma/beta
    fold into adjacent weights. A single explicit ACT-table load
    (natural_log_exp_and_others) serves ln/exp/relu/identity with zero
    mid-kernel ACT_TABLE_LOADs.
  - ALL t<->e layout flips run on the XBAR DMA-transpose engine, batched one
    [128, 4x384] -> [128, 4, 3, 128] op per LN block / attention block, all
    issued from the Sync engine (the xbar is a serial resource: concurrent
    transposes from two queues corrupt - verified on HW). PE transposes and
    their DVE evacuation copies are gone entirely.
  - matmuls in bf16 (fp32 PSUM accumulation).
  - attention: scores for all 6 heads of one (seq, key-chunk) go to a single
    3-bank PSUM tile [128, 6, 256] via 6 matmuls (head pairs run concurrently
    on the PE via base-partition col/row groups), evacuated by ONE wide exp;
    probs are masked multiplicatively after exp. o accumulates per-query-tile
    in a [128, H, 65] PSUM tile with ones-augmented V supplying the softmax
    denominators; evacuation fuses the reciprocal multiply.
  - proj and the MLP second linear produce TOKEN-major outputs directly
    (activation^T chunks as the stationary operand) so the residual add
    consumes PSUM straight.
  - all-zero biases (this model instance) are detected on the host and their
    adds/b-matmuls are skipped entirely.
"""

import numpy as np
import ml_dtypes

BF16 = ml_dtypes.bfloat16

P = 128
T = 256
E = 384
V = 65
H = 6
HS = 64
FF = 1536
L = 3
NCORES = 8
BPC = 16              # sequences per core
TOK = BPC * T         # 4096 tokens per core
NT = TOK // P         # 32 token tiles
NB = TOK // 512       # 8 blocks of 512 tokens (2 seqs)
ECH = E // P          # 3
FCH = FF // P         # 12

_NC_CACHE = {}


def _build_nc(flags):
    """Build + compile the Bass program. flags = (bqk_nz, bv_nz, bp_nz, b1_nz, b2_nz)."""
    import concourse.bacc as bacc
    import concourse.mybir as mybir
    import concourse.tile as tile

    dt = mybir.dt
    f32 = dt.float32
    bf = dt.bfloat16
    Alu = mybir.AluOpType
    Act = mybir.ActivationFunctionType

    import os
    DBG_NO_DMAT = bool(os.environ.get("KBG_NO_DMAT"))
    DBG_NO_WEXP = bool(os.environ.get("KBG_NO_WEXP"))
    DBG_NO_TBL = bool(os.environ.get("KBG_NO_TBL"))
    DBG_BASE_SC = bool(os.environ.get("KBG_BASE_SC"))

    nc = bacc.Bacc("TRN2", target_bir_lowering=False, debug=False, num_devices=1)

    # ---- DRAM tensors (shapes match SBUF layouts; host pre-arranges) ----
    D = {}
    D["oh"] = nc.dram_tensor("oh", [V, TOK], bf, kind="ExternalInput")
    D["te"] = nc.dram_tensor("te", [V, E], bf, kind="ExternalInput")
    D["pos"] = nc.dram_tensor("pos", [P, 2, E], f32, kind="ExternalInput")
    D["mask"] = nc.dram_tensor("mask", [P, 2 * P], bf, kind="ExternalInput")
    for l in range(L):
        for w in ("wq", "wk", "wv", "wproj"):
            D[f"{w}{l}"] = nc.dram_tensor(f"{w}{l}", [P, ECH, E], bf, kind="ExternalInput")
        D[f"bq{l}"] = nc.dram_tensor(f"bq{l}", [P, ECH], f32, kind="ExternalInput")
        D[f"bk{l}"] = nc.dram_tensor(f"bk{l}", [P, ECH], f32, kind="ExternalInput")
        D[f"w1{l}"] = nc.dram_tensor(f"w1{l}", [P, ECH, FF], bf, kind="ExternalInput")
        D[f"b1c{l}"] = nc.dram_tensor(f"b1c{l}", [P, FCH], f32, kind="ExternalInput")
        D[f"w2{l}"] = nc.dram_tensor(f"w2{l}", [P, FCH, E], bf, kind="ExternalInput")
        D[f"bvrow{l}"] = nc.dram_tensor(f"bvrow{l}", [1, E], bf, kind="ExternalInput")
        D[f"bprow{l}"] = nc.dram_tensor(f"bprow{l}", [1, E], bf, kind="ExternalInput")
        D[f"b2row{l}"] = nc.dram_tensor(f"b2row{l}", [1, E], bf, kind="ExternalInput")
    D["wout"] = nc.dram_tensor("wout", [P, ECH, V], bf, kind="ExternalInput")
    D["boutc"] = nc.dram_tensor("boutc", [V, 1], f32, kind="ExternalInput")
    D["logT"] = nc.dram_tensor("logT", [V, TOK], f32, kind="ExternalOutput")

    bqk_nz, bv_nz, bp_nz, b1_nz, b2_nz = flags

    with tile.TileContext(nc) as tc:
        import contextlib

        with contextlib.ExitStack() as ctx:
            # one table load serving ln/exp/relu/identity for the whole kernel
            if not DBG_NO_TBL:
                nc.scalar.add_instruction(
                    mybir.InstLoadActFuncSet(
                        name="I-acttbl", ins=[], outs=[], act_func_set_id=6
                    )
                )

            const = ctx.enter_context(tc.tile_pool(name="const", bufs=1))
            wpool = ctx.enter_context(tc.tile_pool(name="wpool", bufs=2))
            act = ctx.enter_context(tc.tile_pool(name="act", bufs=4))
            act2 = ctx.enter_context(tc.tile_pool(name="act2", bufs=2))
            act1 = ctx.enter_context(tc.tile_pool(name="act1", bufs=1))
            ps_mm = ctx.enter_context(tc.tile_pool(name="ps_mm", bufs=3 if DBG_NO_DMAT else 4, space="PSUM"))
            ps_sc = None if DBG_BASE_SC else ctx.enter_context(tc.tile_pool(name="ps_sc", bufs=1, space="PSUM"))
            ps_o = ctx.enter_context(tc.tile_pool(name="ps_o", bufs=1, space="PSUM"))

            def load_const(name, shape, dtp):
                t = const.tile(shape, dtp, tag=name)
                nc.sync.dma_start(out=t[:], in_=D[name].ap())
                return t

            # pad the K=65 embedding contraction to K=128 (sub-128 partition
            # matmuls are flaky on HW); pad rows are zeroed so they add 0.
            oh_sb = const.tile([P, TOK], bf, tag="oh")
            nc.vector.memset(oh_sb[:], 0.0)
            nc.sync.dma_start(out=oh_sb[0:V, :], in_=D["oh"].ap())
            te_sb = const.tile([P, E], bf, tag="te")
            nc.vector.memset(te_sb[:], 0.0)
            nc.sync.dma_start(out=te_sb[0:V, :], in_=D["te"].ap())
            pos_sb = load_const("pos", [P, 2, E], f32)
            mask_sb = load_const("mask", [P, 2 * P], bf)
            boutc_sb = load_const("boutc", [V, 1], f32)
            ones_sb = const.tile([1, P], bf, tag="ones")
            nc.vector.memset(ones_sb[:], 1.0)
            eps_sb = const.tile([P, 1], f32, tag="eps")
            nc.vector.memset(eps_sb[:], 1e-5)

            def warm():
                """tiny matmul to keep the PE HAM clock-gate at 8/8 across
                attention stall windows; writes a scratch corner of an mm
                ring slot, never read."""
                wt = ps_mm.tile([P, 512], f32, tag="mm", name="warm")
                nc.tensor.matmul(
                    wt[0:8, 0:8], mask_sb[:, 0:8], mask_sb[:, 0:8],
                    start=True, stop=True,
                )
            if DBG_NO_DMAT:
                from concourse.masks import make_identity
                ident_sb = const.tile([P, P], bf, tag="ident")
                make_identity(nc, ident_sb[:])

            def blk_transpose(dst4, src):
                """dst4 [P,4,ECH,P] = per-tile transpose of src [P,4,E]."""
                if not DBG_NO_DMAT:
                    nc.sync.dma_start_transpose(
                        dst4[:], src[:].rearrange("p j e -> p (j e)"))
                    return
                for j in range(4):
                    for c in range(ECH):
                        tp = ps_mm.tile([P, 512], f32, tag="tp", name="tp", bufs=1)
                        tpb = tp[:, 0:64].bitcast(bf)
                        nc.tensor.transpose(
                            tpb, src[:, j, c * P:(c + 1) * P], ident_sb[:])
                        nc.vector.tensor_copy(out=dst4[:, j, c, :], in_=tpb)

            def blk_transpose2(dst4, src2, s):
                """dst4[:, 2s:2s+2] = transpose of src2 [P, 2, E] (one seq)."""
                if not DBG_NO_DMAT:
                    nc.sync.dma_start_transpose(
                        dst4[:, 2 * s:2 * s + 2],
                        src2[:].rearrange("p j e -> p (j e)"))
                    return
                for j in range(2):
                    for c in range(ECH):
                        tp = ps_mm.tile([P, 512], f32, tag="tp", name="tp", bufs=1)
                        tpb = tp[:, 0:64].bitcast(bf)
                        nc.tensor.transpose(
                            tpb, src2[:, j, c * P:(c + 1) * P], ident_sb[:])
                        nc.vector.tensor_copy(out=dst4[:, 2 * s + j, c, :], in_=tpb)

            # two persistent Vt tiles (alternating per block); the ones column
            # (col 64) is set once and never rewritten - per-block evacs only
            # touch cols 0:64.
            vt_tiles = []
            for i in range(2):
                vt = const.tile([P, 4, H, 65], bf, tag=f"Vt{i}", name=f"Vt{i}")
                nc.gpsimd.memset(vt[:, :, :, 64:65], 1.0)
                vt_tiles.append(vt)

            # persistent residual tiles
            h = []
            for i in range(NT):
                h.append(const.tile([P, E], f32, tag=f"h{i}", name=f"h{i}"))

            # ---- embedding: h = onehot.T @ tok_emb + pos ----
            def embed_emit(lo, hi):
                for i in range(lo, hi):
                    ps = ps_mm.tile([P, 512], f32, tag="mm", name="emm")
                    nc.tensor.matmul(
                        ps[:, 0:E], oh_sb[:, i * P:(i + 1) * P], te_sb[:],
                        start=True, stop=True,
                    )
                    nc.vector.tensor_add(
                        out=h[i][:], in0=ps[:, 0:E], in1=pos_sb[:, i % 2, :])

            def ln_block(i0):
                """LN of h[i0..i0+3] -> xnT [P, 4, ECH, 128] bf16 via one
                XBAR DMA transpose of the whole [P, 4, E] block."""
                xn = act2.tile([P, 4, E], bf, tag="xn")
                mv4 = act.tile([P, 4, 2], f32, tag="mv")
                rstd4 = act.tile([P, 4], f32, tag="rstd")
                for j in range(4):
                    st6 = act.tile([P, 6], f32, tag="bnst")
                    nc.vector.bn_stats(out=st6[:], in_=h[i0 + j][:])
                    nc.vector.bn_aggr(out=mv4[:, j, :], in_=st6[:])
                # rstd = exp(-0.5 * ln(var + eps))
                nc.scalar.activation(
                    out=rstd4[:], in_=mv4[:, :, 1], func=Act.Ln, bias=eps_sb[:],
                )
                nc.scalar.activation(
                    out=rstd4[:], in_=rstd4[:], func=Act.Exp, scale=-0.5,
                )
                for j in range(4):
                    nc.vector.tensor_scalar(
                        out=xn[:, j, :], in0=h[i0 + j][:],
                        scalar1=mv4[:, j, 0:1], scalar2=rstd4[:, j:j + 1],
                        op0=Alu.subtract, op1=Alu.mult,
                    )
                xnT = act.tile([P, 4, ECH, P], bf, tag="xnT")
                blk_transpose(xnT, xn)
                return xnT

            def linear_fmaj(xnT, w_sb, bias_sb, fch, tag, relu=False,
                            act_evac=False):
                """feature-major out [P, fch, 512] bf16 = (W^T xn^T);
                bias per-partition (or None). relu/act_evac route evac to
                ScalarE."""
                o = (act1 if fch == FCH else act2).tile([P, fch, 512], bf, tag=tag, name=tag)
                for f in range(fch):
                    ps = ps_mm.tile([P, 512], f32, tag="mm")
                    for c in range(ECH):
                        nc.tensor.matmul(
                            ps[:], w_sb[:, c, f * P:(f + 1) * P], xnT[:, :, c, :],
                            start=(c == 0), stop=(c == ECH - 1),
                        )
                    if relu:
                        if bias_sb is not None:
                            nc.scalar.activation(
                                out=o[:, f, :], in_=ps[:], func=Act.Relu,
                                bias=bias_sb[:, f:f + 1], scale=1.0,
                            )
                        else:
                            nc.scalar.activation(
                                out=o[:, f, :], in_=ps[:], func=Act.Relu, scale=1.0,
                            )
                    elif act_evac:
                        if bias_sb is not None:
                            nc.scalar.activation(
                                out=o[:, f, :], in_=ps[:], func=Act.Identity,
                                bias=bias_sb[:, f:f + 1], scale=1.0,
                            )
                        else:
                            nc.scalar.activation(
                                out=o[:, f, :], in_=ps[:], func=Act.Identity, scale=1.0,
                            )
                    else:
                        if bias_sb is not None:
                            nc.vector.tensor_scalar_add(
                                out=o[:, f, :], in0=ps[:], scalar1=bias_sb[:, f:f + 1],
                            )
                        else:
                            nc.vector.tensor_copy(out=o[:, f, :], in_=ps[:])
                return o

            def linear_tok_resid(xT_slices, w_sb, nch, brow, i0, nj=4):
                """h[i0+j] += x @ W (+ b): token-major PSUM output via xT
                chunks as the stationary operand; residual add reads PSUM.
                xT_slices(j, c) -> stationary [P, P] AP."""
                for j in range(nj):
                    ps = ps_mm.tile([P, 512], f32, tag="mm", name="tokmm")
                    for c in range(nch):
                        nc.tensor.matmul(
                            ps[:, 0:E], xT_slices(j, c), w_sb[:, c, :],
                            start=(c == 0),
                            stop=(c == nch - 1 and brow is None),
                        )
                    if brow is not None:
                        nc.tensor.matmul(
                            ps[:, 0:E], ones_sb[:], brow[:], start=False, stop=True,
                        )
                    nc.vector.tensor_add(
                        out=h[i0 + j][:], in0=h[i0 + j][:], in1=ps[:, 0:E])

            def load_w(name, shape, dtp):
                t = wpool.tile(shape, dtp, tag=name[:-1])  # tag without layer idx
                nc.sync.dma_start(out=t[:], in_=D[name].ap())
                return t

            # ---- transformer layers (software-pipelined emission) ----
            W = {}

            def load_layer(l):
                W[l] = dict(
                    wq=load_w(f"wq{l}", [P, ECH, E], bf),
                    wk=load_w(f"wk{l}", [P, ECH, E], bf),
                    wv=load_w(f"wv{l}", [P, ECH, E], bf),
                    wproj=load_w(f"wproj{l}", [P, ECH, E], bf),
                    bq=load_w(f"bq{l}", [P, ECH], f32) if bqk_nz[l] else None,
                    bk=load_w(f"bk{l}", [P, ECH], f32) if bqk_nz[l] else None,
                    w1=load_w(f"w1{l}", [P, ECH, FF], bf),
                    b1c=load_w(f"b1c{l}", [P, FCH], f32) if b1_nz[l] else None,
                    w2=load_w(f"w2{l}", [P, FCH, E], bf),
                    bvrow=load_w(f"bvrow{l}", [1, E], bf) if bv_nz[l] else None,
                    bprow=load_w(f"bprow{l}", [1, E], bf) if bp_nz[l] else None,
                    b2row=load_w(f"b2row{l}", [1, E], bf) if b2_nz[l] else None,
                )

            def attn_emit(l, b, xnT):
                Wl = W[l]
                wq, wk, wv = Wl["wq"], Wl["wk"], Wl["wv"]
                wproj, bq, bk = Wl["wproj"], Wl["bq"], Wl["bk"]
                bvrow, bprow = Wl["bvrow"], Wl["bprow"]
                i0 = 4 * b
                QT = linear_fmaj(xnT, wq, bq, ECH, "QT", act_evac=True)
                KT = linear_fmaj(xnT, wk, bk, ECH, "KT", act_evac=True)
                # V token-major, ones-augmented: [P, 4, H, 65] (col 64 preset)
                Vt = vt_tiles[b % 2]
                for j in range(4):
                    ps = ps_mm.tile([P, 512], f32, tag="mm")
                    for c in range(ECH):
                        nc.tensor.matmul(
                            ps[:, 0:E], xnT[:, j, c, :], wv[:, c, :],
                            start=(c == 0),
                            stop=(c == ECH - 1 and bvrow is None),
                        )
                    if bvrow is not None:
                        nc.tensor.matmul(
                            ps[:, 0:E], ones_sb[:], bvrow[:], start=False, stop=True,
                        )
                    nc.vector.tensor_copy(
                        out=Vt[:, j, :, 0:64],
                        in_=ps[:, 0:E].rearrange("p (h d) -> p h d", h=H),
                    )

                onorm = act2.tile([P, 4, E], bf, tag="onorm")
                for s in range(2):      # the 2 sequences in this block
                    tb = s * 256        # col offset within the 512 block
                    probs = act2.tile([P, 2, H, 256], bf, tag="probs")
                    for st in range(2):  # s_tile (128 keys each)
                        tlo = 128 if st == 1 else 0
                        if DBG_BASE_SC:
                            for hh in range(H):
                                c, off = divmod(hh * HS, P)
                                scb = ps_mm.tile([P, 512], f32, tag="mm", name="scb")
                                nc.tensor.matmul(
                                    scb[:, 0:256 - tlo],
                                    KT[off:off + HS, c, tb + st * P: tb + (st + 1) * P],
                                    QT[off:off + HS, c, tb + tlo: tb + 256],
                                    start=True, stop=True,
                                )
                                nc.scalar.activation(
                                    out=probs[:, st, hh, tlo:256],
                                    in_=scb[:, 0:256 - tlo],
                                    func=Act.Exp, scale=float(HS) ** -0.5,
                                )
                        else:
                            # scores land in cols 0:256-tlo of slot
                            # 2*(hh%3)+hh//3: concurrently-running row-group
                            # pairs (heads 2k/2k+1 at base partitions 0/64)
                            # must write DIFFERENT psum banks, and matmul
                            # psum writes must stay 1KB-aligned (both
                            # verified on HW - violating either faults).
                            sc = ps_sc.tile([P, H, 256], f32, tag="sc", name="sc")
                            for hh in range(H):
                                c, off = divmod(hh * HS, P)
                                slot = 2 * (hh % 3) + hh // 3
                                nc.tensor.matmul(
                                    sc[:, slot, 0:256 - tlo],
                                    KT[off:off + HS, c, tb + st * P: tb + (st + 1) * P],
                                    QT[off:off + HS, c, tb + tlo: tb + 256],
                                    start=True, stop=True,
                                )
                            # one wide exp for all 6 heads of this
                            # key-chunk; probs stays in SLOT order (the mask
                            # is head-agnostic; o-matmuls index by slot)
                            if DBG_NO_WEXP or st == 1:
                                # st=1 reads are strided across psum banks -
                                # ACT faults on that (HW); per-head reads
                                # stay within a bank.
                                for sl in range(H):
                                    nc.scalar.activation(
                                        out=probs[:, st, sl, tlo:256],
                                        in_=sc[:, sl, 0:256 - tlo],
                                        func=Act.Exp, scale=float(HS) ** -0.5,
                                    )
                            else:
                                nc.scalar.activation(
                                    out=probs[:, st, :, tlo:256],
                                    in_=sc[:, :, 0:256 - tlo],
                                    func=Act.Exp, scale=float(HS) ** -0.5,
                                )
                        if st == 0:
                            nc.vector.tensor_tensor(
                                out=probs[:, 0], in0=probs[:, 0],
                                in1=mask_sb[:, None, :].to_broadcast((P, H, 256)),
                                op=Alu.mult,
                            )
                        else:
                            nc.vector.tensor_tensor(
                                out=probs[:, 1, :, P:256],
                                in0=probs[:, 1, :, P:256],
                                in1=mask_sb[:, None, 0:P].to_broadcast((P, H, P)),
                                op=Alu.mult,
                            )
                    # o-matmuls: all heads into one [P, H, 65] PSUM tile;
                    # the two key chunks accumulate in PSUM; evac fuses the
                    # softmax normalization via one recip + one broadcast mult.
                    for tt in range(2):  # query tiles of this seq
                        osum = ps_o.tile([P, H, 65], f32, tag="osum", name="osum")
                        for hh in range(H):
                            psl = 2 * (hh % 3) + hh // 3 if not DBG_BASE_SC else hh
                            nc.tensor.matmul(
                                osum[:, hh, :],
                                probs[:, 0, psl, tt * P:(tt + 1) * P],
                                Vt[:, 2 * s, hh, :],
                                start=True, stop=(tt == 0),
                            )
                            if tt == 1:
                                nc.tensor.matmul(
                                    osum[:, hh, :],
                                    probs[:, 1, psl, P:2 * P],
                                    Vt[:, 2 * s + 1, hh, :],
                                    start=False, stop=True,
                                )
                        rec = act.tile([P, H], f32, tag="rec", name="rec")
                        nc.vector.reciprocal(out=rec[:], in_=osum[:, :, 64])
                        nc.vector.tensor_tensor(
                            out=onorm[:, 2 * s + tt].rearrange("p (h d) -> p h d", h=H),
                            in0=osum[:, :, 0:64],
                            in1=rec[:, :, None].to_broadcast((P, H, HS)),
                            op=Alu.mult,
                        )
                oT = act2.tile([P, 4, ECH, P], bf, tag="oT")
                blk_transpose(oT, onorm)
                linear_tok_resid(
                    lambda j, c: oT[:, j, c, :], wproj, ECH, bprow, i0)

            def mlp_emit(l, b):
                i0 = 4 * b
                xnT2 = ln_block(i0)
                aT = linear_fmaj(xnT2, W[l]["w1"], W[l]["b1c"], FCH, "aT",
                                 relu=True)
                linear_tok_resid(
                    lambda j, c: aT[:, c, j * P:(j + 1) * P], W[l]["w2"], FCH,
                    W[l]["b2row"], i0)

            wout = wpool.tile([P, ECH, V], bf, tag="wout")
            nc.sync.dma_start(out=wout[:], in_=D["wout"].ap())

            def final_emit(b, xnfT):
                ps = ps_mm.tile([P, 512], f32, tag="mm")
                for c in range(ECH):
                    nc.tensor.matmul(
                        ps[0:V, :], wout[:, c, :], xnfT[:, :, c, :],
                        start=(c == 0), stop=(c == ECH - 1),
                    )
                lt = act2.tile([V, 512], f32, tag="lt")
                nc.vector.tensor_scalar_add(out=lt[:], in0=ps[0:V, :], scalar1=boutc_sb[:])
                nc.sync.dma_start(
                    out=D["logT"].ap()[:, b * 512:(b + 1) * 512], in_=lt[:],
                )

            # stage pipeline: LN for stage i+1 is emitted during stage i,
            # and stage i's MLP trails one stage behind its attention, so
            # the LN chains + DMA transposes hide under PE-heavy stretches.
            load_layer(0)
            if L > 1:
                load_layer(1)
            stages = [(l, b) for l in range(L) for b in range(NB)]
            stages += [(L, b) for b in range(NB)]      # final LN + unembed
            # prologue: embed block 0, start its LN, then the rest
            embed_emit(0, 4)
            xnT_pre = ln_block(0)
            embed_emit(4, NT)
            # per stage, emission (= scheduler priority + Sync-queue order)
            # follows readiness: next-stage LN first (its DMA transpose must
            # not sit behind this stage's late oT transpose on the serial
            # Sync queue), then the trailing MLP as PE fill work, then the
            # current attention (oT transpose last).
            for idx, (l, b) in enumerate(stages):
                if l < L:
                    attn_emit(l, b, xnT_pre)
                else:
                    final_emit(b, xnT_pre)
                if idx > 0 and stages[idx - 1][0] < L:
                    pl, pb = stages[idx - 1]
                    mlp_emit(pl, pb)
                    if pb == NB - 1 and pl + 2 < L:
                        load_layer(pl + 2)
                if idx + 1 < len(stages):
                    xnT_pre = ln_block(4 * stages[idx + 1][1])
            pl, pb = stages[-1]
            if pl < L:
                mlp_emit(pl, pb)

    nc.compile()
    return nc


def _prep_shared(inp):
    """Host-side weight prep: layout rearrangement + LN gamma/beta folding."""
    sh = {}

    def f32(x):
        return np.asarray(x, np.float32)

    sh["te"] = np.asarray(f32(inp["tok_emb"]), BF16)                      # [V,E]
    sh["pos"] = np.ascontiguousarray(
        f32(inp["pos_emb"]).reshape(2, P, E).transpose(1, 0, 2))          # [P,2,E]
    m = np.concatenate(
        [np.triu(np.ones((P, P), np.float32)), np.ones((P, P), np.float32)], axis=1)
    sh["mask"] = np.asarray(m, BF16)                                      # [P,256]

    def tile3(w, fdim):  # [E, fdim] -> [P, ECH, fdim]
        return np.ascontiguousarray(w.reshape(ECH, P, fdim).transpose(1, 0, 2))

    def col(b, nch):  # [nch*P] -> [P, nch]
        return np.ascontiguousarray(b.reshape(nch, P).T)

    bqk_nz, bv_nz, bp_nz, b1_nz, b2_nz = [], [], [], [], []
    for l in range(L):
        g1, b1_ = f32(inp["ln1_g"][l]), f32(inp["ln1_b"][l])
        g2, b2_ = f32(inp["ln2_g"][l]), f32(inp["ln2_b"][l])
        wq = f32(inp["Wq"][l]).transpose(1, 0, 2).reshape(E, E)   # head-major cols
        wk = f32(inp["Wk"][l]).transpose(1, 0, 2).reshape(E, E)
        wv = f32(inp["Wv"][l]).transpose(1, 0, 2).reshape(E, E)
        sh[f"wq{l}"] = np.asarray(tile3(g1[:, None] * wq, E), BF16)
        sh[f"wk{l}"] = np.asarray(tile3(g1[:, None] * wk, E), BF16)
        sh[f"wv{l}"] = np.asarray(tile3(g1[:, None] * wv, E), BF16)
        bq = wq.T @ b1_
        bk = wk.T @ b1_
        sh[f"bq{l}"] = col(bq, ECH)
        sh[f"bk{l}"] = col(bk, ECH)
        bqk_nz.append(bool(np.any(bq != 0) or np.any(bk != 0)))
        bv = wv.T @ b1_
        sh[f"bvrow{l}"] = np.asarray(bv[None, :], BF16)
        bv_nz.append(bool(np.any(bv != 0)))
        wp = f32(inp["Wproj"][l])
        sh[f"wproj{l}"] = np.asarray(tile3(wp, E), BF16)
        bp = f32(inp["bproj"][l])
        sh[f"bprow{l}"] = np.asarray(bp[None, :], BF16)
        bp_nz.append(bool(np.any(bp != 0)))
        w1 = f32(inp["W1"][l])
        sh[f"w1{l}"] = np.asarray(tile3(g2[:, None] * w1, FF), BF16)
        b1c = f32(inp["b1"][l]) + w1.T @ b2_
        sh[f"b1c{l}"] = col(b1c, FCH)
        b1_nz.append(bool(np.any(b1c != 0)))
        w2 = f32(inp["W2"][l])
        sh[f"w2{l}"] = np.asarray(
            w2.reshape(FCH, P, E).transpose(1, 0, 2), BF16)
        b2r = f32(inp["b2"][l])
        sh[f"b2row{l}"] = np.asarray(b2r[None, :], BF16)
        b2_nz.append(bool(np.any(b2r != 0)))

    gf, bf_ = f32(inp["lnf_g"]), f32(inp["lnf_b"])
    wo = f32(inp["Wout"])
    sh["wout"] = np.asarray(tile3(gf[:, None] * wo, V), BF16)
    sh["boutc"] = (f32(inp["bout"]) + wo.T @ bf_).reshape(V, 1)
    flags = (tuple(bqk_nz), tuple(bv_nz), tuple(bp_nz), tuple(b1_nz), tuple(b2_nz))
    return sh, flags


def _onehot(xc):
    """xc: [BPC, T] ints -> [V, TOK] bf16 one-hot (feature-major)."""
    xf = np.asarray(xc, np.int64).reshape(-1)
    oh = np.zeros((V, TOK), np.float32)
    oh[xf, np.arange(TOK)] = 1.0
    return np.asarray(oh, BF16)


def _get_nc(flags):
    import os
    if os.environ.get("KBG_ALLBIAS"):
        flags = tuple(tuple(True for _ in f) for f in flags)
    if flags not in _NC_CACHE:
        _NC_CACHE[flags] = _build_nc(flags)
    return _NC_CACHE[flags]


def make_in_maps(inputs):
    sh, flags = _prep_shared(inputs)
    x = np.asarray(inputs["x"])
    in_maps = []
    for c in range(NCORES):
        m = dict(sh)
        m["oh"] = _onehot(x[c * BPC:(c + 1) * BPC])
        in_maps.append(m)
    return in_maps, flags


def kernel(**inputs):
    import os
    from concourse.bass_utils import run_bass_kernel_spmd

    in_maps, flags = make_in_maps(inputs)
    nc = _get_nc(flags)
    kw = {}
    if os.environ.get("BASS_TRACE"):
        d = os.environ.get("BASS_TRACE_DIR", "/tmp/bass_trace")
        os.makedirs(d, exist_ok=True)
        kw["tmpdir"] = d
    res = None
    last_err = None
    for attempt in range(2):
        try:
            res = run_bass_kernel_spmd(nc, in_maps, list(range(NCORES)), **kw)
            # force materialization so a flaky device fault surfaces here
            for c in range(NCORES):
                np.asarray(res.results[c]["logT"])
            break
        except Exception as e:          # intermittent device-side fault
            last_err = e
            res = None
    if res is None:
        # a faulted device context can poison this process; a fresh
        # subprocess recovers reliably
        out = _run_in_subprocess(inputs)
        if out is not None:
            return out
        raise last_err
    kernel._last = res
    outs = []
    for c in range(NCORES):
        lt = np.asarray(res.results[c]["logT"], np.float32)   # [V, TOK]
        outs.append(np.ascontiguousarray(lt.T).reshape(BPC, T, V))
    return np.concatenate(outs, axis=0)


def _run_in_subprocess(inputs, tries=2):
    import os
    import subprocess
    import sys
    import tempfile

    kdir = os.path.dirname(os.path.abspath(__file__))
    with tempfile.TemporaryDirectory() as td:
        inp = os.path.join(td, "in.npz")
        outp = os.path.join(td, "out.npy")
        np.savez(inp, **{k: np.asarray(v) for k, v in inputs.items()})
        child = (
            "import numpy as np, sys; sys.path.insert(0, %r); "
            "import kernel as K; d = np.load(%r); "
            "out = K.kernel(**{k: d[k] for k in d.files}); "
            "np.save(%r, out)" % (kdir, inp, outp)
        )
        env = dict(os.environ)
        env.pop("BASS_TRACE", None)      # no profiling in recovery runs
        for _ in range(tries):
            try:
                subprocess.run(
                    [sys.executable, "-c", child], env=env, check=True,
                    timeout=1200,
                )
                return np.load(outp)
            except Exception:
                continue
    return None


kernel._last = None
